# revision 32
# baseline (speedup 1.0000x reference)
# Trainium2 Bass kernel for MergedColumnParallelLinearWithTopping
# (base column-parallel GEMM + per-token LoRA "topping", Punica-style).
#
# Sharding: 2 halves x 4 token-groups. Core c = (h, tg) owns tokens
# [tg*512, (tg+1)*512) and output columns [h*5632, (h+1)*5632), so the
# per-token x@A LoRA activation for a (token, half) pair is computed on
# exactly one core -- the pure column-parallel split would compute each
# half's x@A redundantly on 4 cores (~10% of all PE work).
#
# Math per core:
#   out_c = x_tg @ W_h.T + ((x_tg @ A_h) * M_tg) @ B_h
# where A_h = concat_l A_buffer[l,:,half]  [D, L*R]
#       B_h = concat_l B_buffer[l,:,cols]  [L*R, 5632]
#       M[t, l*R+r] = (weight_indices[t] == l)   (host-precomputed one-hot)
# The one-hot mask turns the per-token gather into dense GEMMs that
# accumulate in the same PSUM group as the base GEMM.
#
# All matmul operands are bf16 (same 1 elem/cell/cycle PE rate as float32r
# for moving free dim 512, but half the HBM traffic).  x/A/B/masks are
# SBUF-resident; W streams through a rotating pool (23MB/core, ~64us of
# DMA, hidden under ~176us of PE).  Warm-up matmuls keep the PE clock
# ramped while the first operands stream in.  PSUM accumulation stays fp32.
# Output DMAs issue from the idle Pool engine so the blocked W-load stream
# on the sync queue never delays them.
#
# Self-contained: hardcodes shapes, builds the Bass program, shards inputs,
# runs on cores 0-7 via run_bass_kernel_spmd, reassembles the full output.

import numpy as np

# Problem shapes (hardcoded per spec)
T, D = 2048, 2048
L, R = 16, 16
BDIM = 5632
NCORES = 8
P = 128
KO = D // P               # 16 contraction chunks
TS = 512                  # tokens per core (matmul moving free dim)
NTG = T // TS             # 4 token groups
MCH = BDIM // P           # 44 output-column chunks per core
LR = L * R                # 256 (one half's lora rows)
LRO = LR // P             # 2
NWARM = 28                # PE warm-up matmuls (cover DMA prefix, ramp clock)
WBUFS = 8                 # rotating W tiles in SBUF

_PROGRAM_CACHE = {}


def _build_program():
    import concourse.bacc as bacc
    import concourse.tile as tile
    from concourse import mybir

    f32 = mybir.dt.float32
    bf16 = mybir.dt.bfloat16
    f8 = mybir.dt.float8e4

    nc = bacc.Bacc("TRN2", target_bir_lowering=False, debug=False)

    # All inputs arrive pre-packed on the host into SBUF layout, so every
    # DMA reads/writes long contiguous per-partition runs.  The LoRA path
    # (x@A and the B matmul) runs in fp8e4 with DoubleRow (2x PE rate);
    # the host pre-scales A by 64 and B by 8 and the one-hot mask carries
    # 2^-9 so every fp8 operand sits in the normal range while the
    # delta accumulates into PSUM at its true scale.
    xt_r = nc.dram_tensor("xt", [P, KO, TS], bf16, kind="ExternalInput").ap()
    wt_r = nc.dram_tensor("wt", [MCH, P, KO, P], bf16, kind="ExternalInput").ap()
    ac_r = nc.dram_tensor("ac", [P, KO, LR], f8, kind="ExternalInput").ap()
    bc_r = nc.dram_tensor("bc", [MCH, P, LRO, P], f8, kind="ExternalInput").ap()
    mt_r = nc.dram_tensor("mt", [P, LRO, TS], bf16, kind="ExternalInput").ap()
    out_r = nc.dram_tensor("out", [MCH, P, TS], f32, kind="ExternalOutput").ap()

    with tile.TileContext(nc) as tc:
        with (
            tc.tile_pool(name="consts", bufs=1) as cpool,
            tc.tile_pool(name="wpool", bufs=WBUFS) as wpool,
            tc.tile_pool(name="outp", bufs=3) as outpool,
            tc.tile_pool(name="psout", bufs=4, space="PSUM") as psout,
            tc.tile_pool(name="psxa", bufs=2, space="PSUM") as psxa,
            tc.tile_pool(name="pswarm", bufs=1, space="PSUM") as pswarm,
        ):
            # Split DMAs that feed matmul operands into k-groups: the fused
            # matmul's LDWEIGHTS has very few semaphore-wait slots, so each
            # matmul must depend on at most one small DMA.
            KG = 4  # k-chunks per sub-DMA

            # --- SBUF-resident tiles ------------------------------------
            warm = cpool.tile([P, P], bf16, name="warm")
            a_sb = cpool.tile([P, KO, LR], f8, name="a_sb")
            b_sb = cpool.tile([P, MCH, LRO, P], f8, name="b_sb")
            xam = cpool.tile([P, LRO, TS], f8, name="xam")
            x_sb = cpool.tile([P, KO, TS], bf16, name="x_sb")
            xf_sb = cpool.tile([P, KO, TS], f8, name="xf_sb")
            mt_sb = cpool.tile([P, LRO, TS], bf16, name="mt_sb")

            # --- DMA issue order = consumption order (SP queue, in-order).
            # First the m=0 base group's operands at fine grain (w0/x
            # k-chunks interleaved) so the PE starts within ~3us, then the
            # x@A operands, then the W/B stream.  W loads beyond WBUFS
            # block on the consuming matmul group (WAR) -- nothing else
            # shares the sync queue behind them (outputs go via Pool).
            w_sb = [
                wpool.tile([P, KO, P], bf16, name=f"w{m}", tag="w")
                for m in range(MCH)
            ]

            def w_load(m, kg=None):
                if kg is None:
                    nc.sync.dma_start(w_sb[m][:], wt_r[m])
                else:
                    nc.sync.dma_start(
                        w_sb[m][:, kg:kg + KG, :], wt_r[m, :, kg:kg + KG, :]
                    )

            def b_load(m):
                nc.sync.dma_start(b_sb[:, m, :, :], bc_r[m])

            def x_load(kg, n=KG):
                nc.sync.dma_start(
                    x_sb[:, kg:kg + n, :], xt_r[:, kg:kg + n, :]
                )

            w_load(0, 0)
            for k in range(KG):  # per-k for the earliest chunks
                x_load(k, 1)
            w_load(0, KG)
            x_load(KG)
            w_load(0, 2 * KG)
            w_load(0, 3 * KG)
            w_load(1)
            x_load(2 * KG)
            x_load(3 * KG)
            b_load(0)
            b_load(1)
            for m in range(2, MCH):
                w_load(m)
                b_load(m)
            # small x@A operands on the scalar queue, in parallel with x/W
            for kg in range(0, KO, 2 * KG):
                nc.scalar.dma_start(
                    a_sb[:, kg:kg + 2 * KG, :], ac_r[:, kg:kg + 2 * KG, :]
                )
            nc.scalar.dma_start(mt_sb[:], mt_r[:])

            # --- PE warm-up: junk matmuls into a scratch PSUM bank keep
            # the clock ramped while the first operands stream in.
            nc.vector.memset(warm[:], 0.0)
            pw = pswarm.tile([P, P], f32, name="pw")
            for _ in range(NWARM):
                nc.tensor.matmul(pw[:], lhsT=warm[:], rhs=warm[:],
                                 start=True, stop=True)

            # fp8 copy of x for the x@A matmuls, converted on the (idle)
            # DVE as each x chunk lands
            for kg in range(0, KO, KG):
                nc.vector.tensor_copy(
                    out=xf_sb[:, kg:kg + KG, :], in_=x_sb[:, kg:kg + KG, :]
                )

            # --- compute -------------------------------------------------
            # Each output chunk m is one PSUM group: 16 base matmuls + 2
            # lora matmuls accumulating in the same bank.  The m=0 group's
            # base half issues first (its operands arrive first), the x@A
            # groups run while a_sb streams in, then m=0 finishes and the
            # remaining 43 chunks pipeline against the W stream.
            def base_w(m, ps, lo, hi):
                for k in range(KO):
                    nc.tensor.matmul(
                        ps[:, lo:hi],
                        lhsT=w_sb[m][:, k, :],
                        rhs=x_sb[:, k, lo:hi],
                        start=(k == 0),
                        stop=False,
                    )

            def base_b(m, ps, lo, hi):
                # single DoubleRow matmul: contraction 256 (both lora-row
                # chunks) at 2x PE rate
                nc.tensor.matmul(
                    ps[:, lo:hi],
                    lhsT=b_sb[:, m, :, :],
                    rhs=xam[:, :, lo:hi],
                    start=False,
                    stop=True,
                    perf_mode=mybir.MatmulPerfMode.DoubleRow,
                )

            def drain(m, ps, lo, hi):
                o = outpool.tile([P, hi - lo], f32, name=f"o_{m}_{lo}", tag="o")
                nc.any.tensor_copy(out=o[:], in_=ps[:, lo:hi])
                nc.gpsimd.dma_start(out_r[m, :, lo:hi], o[:])

            ps0 = psout.tile([P, TS], f32, name="ps_0", tag="ps")
            base_w(0, ps0, 0, TS)

            for mp in range(LRO):
                pxa = psxa.tile([P, TS], f32, name=f"pxa_{mp}", tag="pxa")
                for kp in range(0, KO, 2):
                    nc.tensor.matmul(
                        pxa[:],
                        lhsT=a_sb[:, kp:kp + 2, mp * P:(mp + 1) * P],
                        rhs=xf_sb[:, kp:kp + 2, :],
                        start=(kp == 0),
                        stop=(kp == KO - 2),
                        perf_mode=mybir.MatmulPerfMode.DoubleRow,
                    )
                nc.vector.tensor_tensor(
                    xam[:, mp, :],
                    pxa[:],
                    mt_sb[:, mp, :],
                    mybir.AluOpType.mult,
                )

            base_b(0, ps0, 0, TS)
            drain(0, ps0, 0, TS)

            for m in range(1, MCH):
                ps = psout.tile([P, TS], f32, name=f"ps_{m}", tag="ps")
                if m < MCH - 1:
                    base_w(m, ps, 0, TS)
                    base_b(m, ps, 0, TS)
                    drain(m, ps, 0, TS)
                else:
                    # last chunk: two token-halves so the final drain is
                    # half-sized (shorter kernel tail)
                    hs = TS // 2
                    base_w(m, ps, 0, hs)
                    base_b(m, ps, 0, hs)
                    drain(m, ps, 0, hs)
                    base_w(m, ps, hs, TS)
                    base_b(m, ps, hs, TS)
                    drain(m, ps, hs, TS)

    nc.compile()
    return nc


def get_program():
    if "nc" not in _PROGRAM_CACHE:
        _PROGRAM_CACHE["nc"] = _build_program()
    return _PROGRAM_CACHE["nc"]


def make_in_maps(x, W, A_buffer, B_buffer, weight_indices):
    import ml_dtypes
    from concourse import mybir

    bf16 = ml_dtypes.bfloat16
    f8 = mybir.dt.np(mybir.dt.float8e4)
    x = np.ascontiguousarray(np.asarray(x, dtype=np.float32))
    W = np.asarray(W, dtype=np.float32)
    A = np.asarray(A_buffer, dtype=np.float32)
    B = np.asarray(B_buffer, dtype=np.float32)
    wi = np.asarray(weight_indices).astype(np.int64)

    onehot = (wi[None, :] == np.arange(L, dtype=wi.dtype)[:, None])
    # [LRO, P, T]: one-hot mask expanded to the 256 lora rows of one half.
    # Carries 2^-9 (not 1.0) to cancel the fp8 range scaling of A (x64)
    # and B (x8).
    moh = np.repeat(onehot, R, axis=0).reshape(LRO, P, T) * (2.0 ** -9)

    halves = []
    for h in range(2):
        gcols = slice(h * BDIM, (h + 1) * BDIM)
        wt_h = np.ascontiguousarray(
            W[gcols, :].T.reshape(KO, P, MCH, P).transpose(2, 1, 0, 3)
        ).astype(bf16)  # [MCH, P, KO, P]
        ac_h = np.ascontiguousarray(
            A[:, :, h * R:(h + 1) * R]
            .transpose(1, 0, 2).reshape(KO, P, LR).transpose(1, 0, 2)
            * 64.0
        ).astype(f8)  # [P, KO, LR]
        bc_h = np.ascontiguousarray(
            B[:, :, gcols].reshape(LRO, P, MCH, P).transpose(2, 1, 0, 3)
            * 8.0
        ).astype(f8)  # [MCH, P, LRO, P]
        halves.append((wt_h, ac_h, bc_h))

    in_maps = []
    for c in range(NCORES):
        h, tg = divmod(c, NTG)
        wt_h, ac_h, bc_h = halves[h]
        toks = slice(tg * TS, (tg + 1) * TS)
        xt_f = np.ascontiguousarray(
            x[toks].T.reshape(KO, P, TS).transpose(1, 0, 2)
        )  # [P, KO, TS]
        mt_c = np.ascontiguousarray(
            moh[:, :, toks].transpose(1, 0, 2)
        ).astype(bf16)  # [P, LRO, TS]
        in_maps.append(
            {"xt": xt_f.astype(bf16), "wt": wt_h,
             "ac": ac_h, "bc": bc_h, "mt": mt_c}
        )
    return in_maps


def assemble_output(results):
    out = np.empty((T, 2 * BDIM), dtype=np.float32)
    for c in range(NCORES):
        h, tg = divmod(c, NTG)
        # [MCH, P, TS] -> [tok, col]
        piece = results[c]["out"].transpose(2, 0, 1).reshape(TS, BDIM)
        out[tg * TS:(tg + 1) * TS, h * BDIM:(h + 1) * BDIM] = piece
    return out


def kernel(x, W, A_buffer, B_buffer, weight_indices):
    from concourse.bass_utils import run_bass_kernel_spmd

    in_maps = make_in_maps(x, W, A_buffer, B_buffer, weight_indices)
    nc = get_program()
    res = run_bass_kernel_spmd(
        nc, in_maps, core_ids=list(range(NCORES)), trace=False
    )
    return assemble_output(res.results)


def _make_runner(nc, donate=True):
    """Build a jitted 8-core runner (mirrors bass2jax.run_bass_via_pjrt).
    With donate=False, inputs/zero-outs stay device-resident across calls,
    so repeated calls re-execute the NEFF without re-uploading data."""
    import jax
    import concourse.mybir as mybir
    from jax.sharding import Mesh, NamedSharding, PartitionSpec
    from jax.experimental.shard_map import shard_map
    from concourse.bass2jax import (
        _bass_exec_p,
        install_neuronx_cc_hook,
        partition_id_tensor,
    )

    install_neuronx_cc_hook()

    partition_name = (
        nc.partition_id_tensor.name if nc.partition_id_tensor else None
    )
    in_names, out_names, out_avals, zero_outs = [], [], [], []
    for alloc in nc.m.functions[0].allocations:
        if not isinstance(alloc, mybir.MemoryLocationSet):
            continue
        name = alloc.memorylocations[0].name
        if alloc.kind == "ExternalInput":
            if name != partition_name:
                in_names.append(name)
        elif alloc.kind == "ExternalOutput":
            out_names.append(name)
            shape = tuple(alloc.tensor_shape)
            dtype = mybir.dt.np(alloc.dtype)
            out_avals.append(jax.core.ShapedArray(shape, dtype))
            zero_outs.append(np.zeros(shape, dtype))
    n_params = len(in_names)
    n_outs = len(out_avals)
    all_names = list(in_names) + list(out_names)
    if partition_name is not None:
        all_names.append(partition_name)
    all_names = tuple(all_names)

    def _body(*args):
        operands = list(args)
        if partition_name is not None:
            operands.append(partition_id_tensor())
        outs = _bass_exec_p.bind(
            *operands,
            out_avals=tuple(out_avals),
            in_names=all_names,
            out_names=tuple(out_names),
            lowering_input_output_aliases=(),
            sim_require_finite=True,
            sim_require_nnan=True,
            nc=nc,
        )
        return tuple(outs)

    devices = jax.devices()[:NCORES]
    mesh = Mesh(np.asarray(devices), ("core",))
    in_specs = (PartitionSpec("core"),) * (n_params + n_outs)
    out_specs = (PartitionSpec("core"),) * n_outs
    sharded = jax.jit(
        shard_map(
            _body, mesh=mesh, in_specs=in_specs, out_specs=out_specs,
            check_rep=False,
        ),
        donate_argnums=(
            tuple(range(n_params, n_params + n_outs)) if donate else ()
        ),
        keep_unused=True,
    )

    sharding = NamedSharding(mesh, PartitionSpec("core"))

    def put(in_maps):
        import jax
        concat_in = [
            np.concatenate([in_maps[c][name] for c in range(NCORES)], axis=0)
            for name in in_names
        ]
        concat_zeros = [
            np.zeros((NCORES * z.shape[0], *z.shape[1:]), z.dtype)
            for z in zero_outs
        ]
        return [jax.device_put(a, sharding) for a in concat_in + concat_zeros]

    def unpack(out_arrs):
        return [
            {
                name: np.asarray(out_arrs[i]).reshape(
                    NCORES, *out_avals[i].shape
                )[c]
                for i, name in enumerate(out_names)
            }
            for c in range(NCORES)
        ]

    return sharded, put, unpack


def bench(x, W, A_buffer, B_buffer, weight_indices, iters=24):
    """Returns (output, per_exec_ns, info). Fires `iters` async executions
    with device-resident inputs and blocks at the end, so per-call dispatch
    overlaps execution; the amortized delta approximates HW exec time."""
    import time

    import jax

    in_maps = make_in_maps(x, W, A_buffer, B_buffer, weight_indices)
    nc = get_program()
    sharded, put, unpack = _make_runner(nc, donate=False)
    dev_args = put(in_maps)

    outs = jax.block_until_ready(sharded(*dev_args))  # compile + warm-up
    results = unpack(outs)

    def burst(k):
        t0 = time.monotonic()
        rs = [sharded(*dev_args) for _ in range(k)]
        jax.block_until_ready(rs)
        return time.monotonic() - t0

    burst(2)  # extra warm-up
    t_small = min(burst(2) for _ in range(3))
    t_big = min(burst(2 + iters) for _ in range(3))
    per_exec_ns = (t_big - t_small) / iters * 1e9
    info = {
        "t_small_s": t_small,
        "t_big_s": t_big,
        "iters": iters,
        "per_exec_ns": per_exec_ns,
    }
    return assemble_output(results), per_exec_ns, info


# revision 33
# speedup vs baseline: 3.9243x; 3.9243x over previous
# Trainium2 Bass kernel for MergedColumnParallelLinearWithTopping
# (base column-parallel GEMM + per-token LoRA "topping", Punica-style).
#
# Sharding: 2 halves x 4 token-groups. Core c = (h, tg) owns tokens
# [tg*512, (tg+1)*512) and output columns [h*5632, (h+1)*5632), so the
# per-token x@A LoRA activation for a (token, half) pair is computed on
# exactly one core -- the pure column-parallel split would compute each
# half's x@A redundantly on 4 cores (~10% of all PE work).
#
# Math per core:
#   out_c = x_tg @ W_h.T + ((x_tg @ A_h) * M_tg) @ B_h
# where A_h = concat_l A_buffer[l,:,half]  [D, L*R]
#       B_h = concat_l B_buffer[l,:,cols]  [L*R, 5632]
#       M[t, l*R+r] = (weight_indices[t] == l)   (host-precomputed one-hot)
# The one-hot mask turns the per-token gather into dense GEMMs that
# accumulate in the same PSUM group as the base GEMM.
#
# All matmul operands are bf16 (same 1 elem/cell/cycle PE rate as float32r
# for moving free dim 512, but half the HBM traffic).  x/A/B/masks are
# SBUF-resident; W streams through a rotating pool (23MB/core, ~64us of
# DMA, hidden under ~176us of PE).  Warm-up matmuls keep the PE clock
# ramped while the first operands stream in.  PSUM accumulation stays fp32.
# Output DMAs issue from the idle Pool engine so the blocked W-load stream
# on the sync queue never delays them.
#
# Self-contained: hardcodes shapes, builds the Bass program, shards inputs,
# runs on cores 0-7 via run_bass_kernel_spmd, reassembles the full output.

import numpy as np

# Problem shapes (hardcoded per spec)
T, D = 2048, 2048
L, R = 16, 16
BDIM = 5632
NCORES = 8
P = 128
KO = D // P               # 16 contraction chunks
TS = 512                  # tokens per core (matmul moving free dim)
NTG = T // TS             # 4 token groups
MCH = BDIM // P           # 44 output-column chunks per core
LR = L * R                # 256 (one half's lora rows)
LRO = LR // P             # 2
NWARM = 24                # PE warm-up matmuls (cover DMA prefix, ramp clock)
WBUFS = 8                 # rotating W tiles in SBUF

_PROGRAM_CACHE = {}


def _build_program():
    import concourse.bacc as bacc
    import concourse.tile as tile
    from concourse import mybir

    f32 = mybir.dt.float32
    bf16 = mybir.dt.bfloat16
    f8 = mybir.dt.float8e4

    nc = bacc.Bacc("TRN2", target_bir_lowering=False, debug=False)

    # All inputs arrive pre-packed on the host into SBUF layout, so every
    # DMA reads/writes long contiguous per-partition runs.  The LoRA path
    # (x@A and the B matmul) runs in fp8e4 with DoubleRow (2x PE rate);
    # the host pre-scales A by 64 and B by 8 and the one-hot mask carries
    # 2^-9 so every fp8 operand sits in the normal range while the
    # delta accumulates into PSUM at its true scale.
    xt_r = nc.dram_tensor("xt", [P, KO, TS], bf16, kind="ExternalInput").ap()
    wt_r = nc.dram_tensor("wt", [MCH, P, KO, P], bf16, kind="ExternalInput").ap()
    ac_r = nc.dram_tensor("ac", [P, KO, LR], f8, kind="ExternalInput").ap()
    bc_r = nc.dram_tensor("bc", [MCH, P, LRO, P], f8, kind="ExternalInput").ap()
    mt_r = nc.dram_tensor("mt", [P, LRO, TS], bf16, kind="ExternalInput").ap()
    out_r = nc.dram_tensor("out", [MCH, P, TS], f32, kind="ExternalOutput").ap()

    with tile.TileContext(nc) as tc:
        with (
            tc.tile_pool(name="consts", bufs=1) as cpool,
            tc.tile_pool(name="wpool", bufs=WBUFS) as wpool,
            tc.tile_pool(name="outp", bufs=3) as outpool,
            tc.tile_pool(name="psout", bufs=4, space="PSUM") as psout,
            tc.tile_pool(name="psxa", bufs=2, space="PSUM") as psxa,
            tc.tile_pool(name="pstail", bufs=2, space="PSUM") as pstail,
        ):
            # Split DMAs that feed matmul operands into k-groups: the fused
            # matmul's LDWEIGHTS has very few semaphore-wait slots, so each
            # matmul must depend on at most one small DMA.
            KG = 4  # k-chunks per sub-DMA

            # --- SBUF-resident tiles ------------------------------------
            warm = cpool.tile([P, P], bf16, name="warm")
            a_sb = cpool.tile([P, KO, LR], f8, name="a_sb")
            b_sb = cpool.tile([P, MCH, LRO, P], f8, name="b_sb")
            xam = cpool.tile([P, LRO, TS], f8, name="xam")
            x_sb = cpool.tile([P, KO, TS], bf16, name="x_sb")
            xf_sb = cpool.tile([P, KO, TS], f8, name="xf_sb")
            mt_sb = cpool.tile([P, LRO, TS], bf16, name="mt_sb")

            # --- DMA issue order = consumption order.  The sync queue
            # carries the m=0 base group's operands at fine grain (w0/x
            # k-chunks interleaved) so the PE starts within ~3us, then the
            # W/B stream; the scalar (ACT) queue carries the small x@A
            # operands in parallel.  W loads beyond WBUFS block on the
            # consuming matmul group (WAR) -- nothing else shares the sync
            # queue behind them (outputs go via Pool).
            w_sb = [
                wpool.tile([P, KO, P], bf16, name=f"w{m}", tag="w")
                for m in range(MCH)
            ]

            def w_load(m, kg=None):
                if kg is None:
                    nc.sync.dma_start(w_sb[m][:], wt_r[m])
                else:
                    nc.sync.dma_start(
                        w_sb[m][:, kg:kg + KG, :], wt_r[m, :, kg:kg + KG, :]
                    )

            def b_load(m):
                nc.sync.dma_start(b_sb[:, m, :, :], bc_r[m])

            def x_load(kg, n=KG):
                nc.sync.dma_start(
                    x_sb[:, kg:kg + n, :], xt_r[:, kg:kg + n, :]
                )

            w_load(0, 0)
            for k in range(KG):  # per-k for the earliest chunks
                x_load(k, 1)
            w_load(0, KG)
            x_load(KG)
            w_load(0, 2 * KG)
            w_load(0, 3 * KG)
            w_load(1)
            x_load(2 * KG)
            x_load(3 * KG)
            b_load(0)
            b_load(1)
            for m in range(2, MCH):
                w_load(m)
                b_load(m)
            # small x@A operands on the scalar queue, in parallel with x/W
            for kg in range(0, KO, 2 * KG):
                nc.scalar.dma_start(
                    a_sb[:, kg:kg + 2 * KG, :], ac_r[:, kg:kg + 2 * KG, :]
                )
            nc.scalar.dma_start(mt_sb[:], mt_r[:])

            # --- PE warm-up: junk matmuls into a scratch PSUM bank keep
            # the clock ramped while the first operands stream in.
            nc.vector.memset(warm[:], 0.0)
            pw = pstail.tile([P, P], f32, name="pw", tag="pst")
            for _ in range(NWARM):
                nc.tensor.matmul(pw[:], lhsT=warm[:], rhs=warm[:],
                                 start=True, stop=True)

            # fp8 copy of x for the x@A matmuls, converted on the (idle)
            # DVE as each x chunk lands
            for kg in range(0, KO, KG):
                nc.vector.tensor_copy(
                    out=xf_sb[:, kg:kg + KG, :], in_=x_sb[:, kg:kg + KG, :]
                )

            # --- compute -------------------------------------------------
            # Each output chunk m is one PSUM group: 16 base matmuls + 2
            # lora matmuls accumulating in the same bank.  The m=0 group's
            # base half issues first (its operands arrive first), the x@A
            # groups run while a_sb streams in, then m=0 finishes and the
            # remaining 43 chunks pipeline against the W stream.
            def base_w(m, ps, lo, hi, full=True):
                po = ps[:, lo:hi] if full else ps[:]
                for k in range(KO):
                    nc.tensor.matmul(
                        po,
                        lhsT=w_sb[m][:, k, :],
                        rhs=x_sb[:, k, lo:hi],
                        start=(k == 0),
                        stop=False,
                    )

            def base_b(m, ps, lo, hi, full=True):
                # single DoubleRow matmul: contraction 256 (both lora-row
                # chunks) at 2x PE rate
                nc.tensor.matmul(
                    ps[:, lo:hi] if full else ps[:],
                    lhsT=b_sb[:, m, :, :],
                    rhs=xam[:, :, lo:hi],
                    start=False,
                    stop=True,
                    perf_mode=mybir.MatmulPerfMode.DoubleRow,
                )

            def drain(m, ps, lo, hi, full=True):
                o = outpool.tile([P, hi - lo], f32, name=f"o_{m}_{lo}", tag="o")
                nc.any.tensor_copy(out=o[:], in_=ps[:, lo:hi] if full else ps[:])
                nc.gpsimd.dma_start(out_r[m, :, lo:hi], o[:])

            ps0 = psout.tile([P, TS], f32, name="ps_0", tag="ps")
            base_w(0, ps0, 0, TS)

            for mp in range(LRO):
                pxa = psxa.tile([P, TS], f32, name=f"pxa_{mp}", tag="pxa")
                for kp in range(0, KO, 2):
                    nc.tensor.matmul(
                        pxa[:],
                        lhsT=a_sb[:, kp:kp + 2, mp * P:(mp + 1) * P],
                        rhs=xf_sb[:, kp:kp + 2, :],
                        start=(kp == 0),
                        stop=(kp == KO - 2),
                        perf_mode=mybir.MatmulPerfMode.DoubleRow,
                    )
                nc.vector.tensor_tensor(
                    xam[:, mp, :],
                    pxa[:],
                    mt_sb[:, mp, :],
                    mybir.AluOpType.mult,
                )

            base_b(0, ps0, 0, TS)
            drain(0, ps0, 0, TS)

            for m in range(1, MCH):
                if m < MCH - 1:
                    ps = psout.tile([P, TS], f32, name=f"ps_{m}", tag="ps")
                    base_w(m, ps, 0, TS)
                    base_b(m, ps, 0, TS)
                    drain(m, ps, 0, TS)
                else:
                    # last chunk: four token-quarters in separate PSUM
                    # tiles so earlier quarters' drains overlap later
                    # quarters' matmuls and the final drain is
                    # quarter-sized (shorter kernel tail)
                    qs = TS // 4
                    for q in range(4):
                        pq = pstail.tile([P, qs], f32, name=f"pst_{q}",
                                         tag="pst")
                        base_w(m, pq, q * qs, (q + 1) * qs, full=False)
                        base_b(m, pq, q * qs, (q + 1) * qs, full=False)
                        drain(m, pq, q * qs, (q + 1) * qs, full=False)

    nc.compile()
    return nc


def get_program():
    if "nc" not in _PROGRAM_CACHE:
        _PROGRAM_CACHE["nc"] = _build_program()
    return _PROGRAM_CACHE["nc"]


def make_in_maps(x, W, A_buffer, B_buffer, weight_indices):
    import ml_dtypes
    from concourse import mybir

    bf16 = ml_dtypes.bfloat16
    f8 = mybir.dt.np(mybir.dt.float8e4)
    x = np.ascontiguousarray(np.asarray(x, dtype=np.float32))
    W = np.asarray(W, dtype=np.float32)
    A = np.asarray(A_buffer, dtype=np.float32)
    B = np.asarray(B_buffer, dtype=np.float32)
    wi = np.asarray(weight_indices).astype(np.int64)

    onehot = (wi[None, :] == np.arange(L, dtype=wi.dtype)[:, None])
    # [LRO, P, T]: one-hot mask expanded to the 256 lora rows of one half.
    # Carries 2^-9 (not 1.0) to cancel the fp8 range scaling of A (x64)
    # and B (x8).
    moh = np.repeat(onehot, R, axis=0).reshape(LRO, P, T) * (2.0 ** -9)

    halves = []
    for h in range(2):
        gcols = slice(h * BDIM, (h + 1) * BDIM)
        wt_h = np.ascontiguousarray(
            W[gcols, :].T.reshape(KO, P, MCH, P).transpose(2, 1, 0, 3)
        ).astype(bf16)  # [MCH, P, KO, P]
        ac_h = np.ascontiguousarray(
            A[:, :, h * R:(h + 1) * R]
            .transpose(1, 0, 2).reshape(KO, P, LR).transpose(1, 0, 2)
            * 64.0
        ).astype(f8)  # [P, KO, LR]
        bc_h = np.ascontiguousarray(
            B[:, :, gcols].reshape(LRO, P, MCH, P).transpose(2, 1, 0, 3)
            * 8.0
        ).astype(f8)  # [MCH, P, LRO, P]
        halves.append((wt_h, ac_h, bc_h))

    in_maps = []
    for c in range(NCORES):
        h, tg = divmod(c, NTG)
        wt_h, ac_h, bc_h = halves[h]
        toks = slice(tg * TS, (tg + 1) * TS)
        xt_f = np.ascontiguousarray(
            x[toks].T.reshape(KO, P, TS).transpose(1, 0, 2)
        )  # [P, KO, TS]
        mt_c = np.ascontiguousarray(
            moh[:, :, toks].transpose(1, 0, 2)
        ).astype(bf16)  # [P, LRO, TS]
        in_maps.append(
            {"xt": xt_f.astype(bf16), "wt": wt_h,
             "ac": ac_h, "bc": bc_h, "mt": mt_c}
        )
    return in_maps


def assemble_output(results):
    out = np.empty((T, 2 * BDIM), dtype=np.float32)
    for c in range(NCORES):
        h, tg = divmod(c, NTG)
        # [MCH, P, TS] -> [tok, col]
        piece = results[c]["out"].transpose(2, 0, 1).reshape(TS, BDIM)
        out[tg * TS:(tg + 1) * TS, h * BDIM:(h + 1) * BDIM] = piece
    return out


def kernel(x, W, A_buffer, B_buffer, weight_indices):
    from concourse.bass_utils import run_bass_kernel_spmd

    in_maps = make_in_maps(x, W, A_buffer, B_buffer, weight_indices)
    nc = get_program()
    res = run_bass_kernel_spmd(
        nc, in_maps, core_ids=list(range(NCORES)), trace=False
    )
    return assemble_output(res.results)


def _make_runner(nc, donate=True):
    """Build a jitted 8-core runner (mirrors bass2jax.run_bass_via_pjrt).
    With donate=False, inputs/zero-outs stay device-resident across calls,
    so repeated calls re-execute the NEFF without re-uploading data."""
    import jax
    import concourse.mybir as mybir
    from jax.sharding import Mesh, NamedSharding, PartitionSpec
    from jax.experimental.shard_map import shard_map
    from concourse.bass2jax import (
        _bass_exec_p,
        install_neuronx_cc_hook,
        partition_id_tensor,
    )

    install_neuronx_cc_hook()

    partition_name = (
        nc.partition_id_tensor.name if nc.partition_id_tensor else None
    )
    in_names, out_names, out_avals, zero_outs = [], [], [], []
    for alloc in nc.m.functions[0].allocations:
        if not isinstance(alloc, mybir.MemoryLocationSet):
            continue
        name = alloc.memorylocations[0].name
        if alloc.kind == "ExternalInput":
            if name != partition_name:
                in_names.append(name)
        elif alloc.kind == "ExternalOutput":
            out_names.append(name)
            shape = tuple(alloc.tensor_shape)
            dtype = mybir.dt.np(alloc.dtype)
            out_avals.append(jax.core.ShapedArray(shape, dtype))
            zero_outs.append(np.zeros(shape, dtype))
    n_params = len(in_names)
    n_outs = len(out_avals)
    all_names = list(in_names) + list(out_names)
    if partition_name is not None:
        all_names.append(partition_name)
    all_names = tuple(all_names)

    def _body(*args):
        operands = list(args)
        if partition_name is not None:
            operands.append(partition_id_tensor())
        outs = _bass_exec_p.bind(
            *operands,
            out_avals=tuple(out_avals),
            in_names=all_names,
            out_names=tuple(out_names),
            lowering_input_output_aliases=(),
            sim_require_finite=True,
            sim_require_nnan=True,
            nc=nc,
        )
        return tuple(outs)

    devices = jax.devices()[:NCORES]
    mesh = Mesh(np.asarray(devices), ("core",))
    in_specs = (PartitionSpec("core"),) * (n_params + n_outs)
    out_specs = (PartitionSpec("core"),) * n_outs
    sharded = jax.jit(
        shard_map(
            _body, mesh=mesh, in_specs=in_specs, out_specs=out_specs,
            check_rep=False,
        ),
        donate_argnums=(
            tuple(range(n_params, n_params + n_outs)) if donate else ()
        ),
        keep_unused=True,
    )

    sharding = NamedSharding(mesh, PartitionSpec("core"))

    def put(in_maps):
        import jax
        concat_in = [
            np.concatenate([in_maps[c][name] for c in range(NCORES)], axis=0)
            for name in in_names
        ]
        concat_zeros = [
            np.zeros((NCORES * z.shape[0], *z.shape[1:]), z.dtype)
            for z in zero_outs
        ]
        return [jax.device_put(a, sharding) for a in concat_in + concat_zeros]

    def unpack(out_arrs):
        return [
            {
                name: np.asarray(out_arrs[i]).reshape(
                    NCORES, *out_avals[i].shape
                )[c]
                for i, name in enumerate(out_names)
            }
            for c in range(NCORES)
        ]

    return sharded, put, unpack


def bench(x, W, A_buffer, B_buffer, weight_indices, iters=24):
    """Returns (output, per_exec_ns, info). Fires `iters` async executions
    with device-resident inputs and blocks at the end, so per-call dispatch
    overlaps execution; the amortized delta approximates HW exec time."""
    import time

    import jax

    in_maps = make_in_maps(x, W, A_buffer, B_buffer, weight_indices)
    nc = get_program()
    sharded, put, unpack = _make_runner(nc, donate=False)
    dev_args = put(in_maps)

    outs = jax.block_until_ready(sharded(*dev_args))  # compile + warm-up
    results = unpack(outs)

    def burst(k):
        t0 = time.monotonic()
        rs = [sharded(*dev_args) for _ in range(k)]
        jax.block_until_ready(rs)
        return time.monotonic() - t0

    burst(2)  # extra warm-up
    t_small = min(burst(2) for _ in range(3))
    t_big = min(burst(2 + iters) for _ in range(3))
    per_exec_ns = (t_big - t_small) / iters * 1e9
    info = {
        "t_small_s": t_small,
        "t_big_s": t_big,
        "iters": iters,
        "per_exec_ns": per_exec_ns,
    }
    return assemble_output(results), per_exec_ns, info
